# revision 1
# baseline (speedup 1.0000x reference)
"""Trainium2 Bass kernel for the 27092653703365 contrastive loss.

Strategy (memory-bound problem, ~138 MB of image features dominates):
  - Data-parallel shard of the batch dim (bs=256) across 8 NeuronCores
    (32 images per core); random_text_features replicated.
  - Per core: stream the [32, 264, 512] image shard through SBUF once.
    For every image row we need only (a) its dot with ONE text vector and
    (b) its squared norm:
       * dots  -> DVE fused scalar_tensor_tensor (mult + accumulate)
       * norms -> ACT activation(Square, accum_out)
    so DVE and ACT each touch every element exactly once and both stay at
    or under the HBM DMA roofline.
  - The [256 x 32] image-text logits block (columns owned by this core) plus
    the core's partial row-CE sum are AllGathered (8 x 33 KB), after which
    every core finishes the transposed-CE term and the final scalar loss.
"""

import sys

sys.path.insert(0, "/opt/trn_rl_repo")

from contextlib import ExitStack

import numpy as np

import concourse.bass as bass
import concourse.tile as tile
from concourse import mybir
from concourse.bass_utils import run_bass_kernel_spmd

F32 = mybir.dt.float32
AF = mybir.ActivationFunctionType
ALU = mybir.AluOpType

NCORES = 8
BS, FTN, D = 256, 8, 512
ATN = BS + FTN  # 264
BPC = BS // NCORES  # 32 images per core
G = 8  # images per DMA/compute group (two 2 MiB DMAs per group)
NG = BPC // G


def _cap_sync_waits(nc: bass.Bass, max_waits: int = 1) -> None:
    """The walrus build in this container encodes at most one sync-wait
    command per instruction ("Too many sync wait commands" in codegen
    otherwise), but Tile freely attaches several. Splitting the surplus
    waits onto single-wait Drain carriers right before the instruction is
    semantically identical: the engine blocks on each in turn.
    """
    for func in nc.m.functions:
        for bb in func.blocks:
            out = []
            for ins in bb.instructions:
                si = ins.sync_info
                if si is not None and len(si.on_wait) > max_waits:
                    waits = list(si.on_wait)
                    extra, keep = waits[:-max_waits], waits[-max_waits:]
                    for k, w in enumerate(extra):
                        d = mybir.InstDrain(
                            name=f"{ins.name}_w{k}",
                            ins=[],
                            outs=[],
                            engine=ins.engine,
                        )
                        d.sync_info = mybir.SyncInfo(on_wait=[w], on_update=[])
                        nc.register_instruction(d, overwrite=True)
                        out.append(d)
                    ins.sync_info = mybir.SyncInfo(
                        on_wait=keep, on_update=list(si.on_update)
                    )
                out.append(ins)
            bb.instructions = out


def build_nc() -> bass.Bass:
    nc = bass.Bass(num_devices=NCORES)

    img = nc.declare_dram_parameter("img", [BPC, ATN, D], F32, isOutput=False)
    rand = nc.declare_dram_parameter("rand", [BS, D], F32, isOutput=False)
    falset = nc.declare_dram_parameter("falset", [BPC, FTN, D], F32, isOutput=False)
    lscale = nc.declare_dram_parameter("lscale", [1], F32, isOutput=False)
    eye = nc.declare_dram_parameter("eye", [BS, BS], F32, isOutput=False)
    rowmask = nc.declare_dram_parameter("rowmask", [BPC, ATN], F32, isOutput=False)
    loss_out = nc.declare_dram_parameter("loss_out", [1], F32, isOutput=True)

    with tile.TileContext(nc) as tc, ExitStack() as ctx:
        singles = ctx.enter_context(tc.tile_pool(name="singles", bufs=1))
        imgpool = ctx.enter_context(tc.tile_pool(name="img", bufs=2))
        tmppool = ctx.enter_context(tc.tile_pool(name="tmp", bufs=2))
        small = ctx.enter_context(tc.tile_pool(name="small", bufs=2))
        psum = ctx.enter_context(tc.tile_pool(name="psum", bufs=2, space="PSUM"))
        dram = ctx.enter_context(tc.tile_pool(name="dram", bufs=1, space="DRAM"))

        # ---- resident tiles -------------------------------------------------
        # rand text, a-chunked: rand2[p, c, d] = rand[c*128+p, d]
        rand2 = singles.tile([128, 2, D], F32)
        nc.gpsimd.dma_start(out=rand2, in_=rand[:, :].rearrange("(c p) d -> p c d", p=128))
        # eye chunks for the transposed-CE diagonal: eye2[p, c, q] = eye[c*128+p, q]
        eye2 = singles.tile([128, 2, BS], F32)
        nc.gpsimd.dma_start(out=eye2, in_=eye[:, :].rearrange("(c p) q -> p c q", p=128))
        # 128x128 identity for PE transpose
        id128 = singles.tile([128, 128], F32)
        nc.gpsimd.dma_start(out=id128, in_=eye[0:128, 0:128])
        # per-core row-CE diagonal mask [32, 264]
        rmask = singles.tile([BPC, ATN], F32)
        nc.gpsimd.dma_start(out=rmask, in_=rowmask[:, :])
        # false texts + image tail rows, both [b=32, f=8, d]
        false_t = singles.tile([BPC, FTN, D], F32)
        nc.gpsimd.dma_start(out=false_t, in_=falset[:, :, :])
        tail_t = singles.tile([BPC, FTN, D], F32)
        nc.gpsimd.dma_start(out=tail_t, in_=img[:, BS:ATN, :])
        # exp(logit_scale) broadcast to all partitions
        ls_raw = singles.tile([128, 1], F32)
        nc.gpsimd.dma_start(out=ls_raw, in_=lscale[:].to_broadcast([128, 1]))
        scale_b = singles.tile([128, 1], F32)
        nc.scalar.activation(scale_b, ls_raw, AF.Exp)
        ones128 = singles.tile([128, 1], F32)
        nc.vector.memset(ones128, 1.0)

        # accumulators: col j = c*32 + b_local
        dots01 = singles.tile([128, 2 * BPC], F32)
        nsq01 = singles.tile([128, 2 * BPC], F32)
        dots_f = singles.tile([BPC, FTN], F32)
        nsq_f = singles.tile([BPC, FTN], F32)
        nsq_ft = singles.tile([BPC, FTN], F32)


        # ---- main stream: dots + squared norms for a-chunks 0/1 -------------
        for g in range(NG):
            b0 = g * G
            img_t = imgpool.tile([128, G, 2, D], F32)
            # One 3-dim DMA per a-chunk (4-dim APs are rejected); two HWDGE
            # rings (SP + ACT) so the chunk loads stream in parallel.
            src = img[b0 : b0 + G, 0:BS, :].rearrange("g (c p) d -> p c g d", p=128)
            nc.sync.dma_start(out=img_t[:, :, 0, :], in_=src[:, 0])
            nc.scalar.dma_start(out=img_t[:, :, 1, :], in_=src[:, 1])
            for i in range(G):
                for c in range(2):
                    col = c * BPC + b0 + i
                    prod = tmppool.tile([128, D], F32, tag="prod")
                    nc.vector.scalar_tensor_tensor(
                out=prod,
                in0=img_t[:, i, c, :],
                scalar=1.0,
                in1=rand2[:, c, :],
                op0=ALU.mult,
                op1=ALU.mult,
                accum_out=dots01[:, col : col + 1],
            )
                    sq = tmppool.tile([128, D], F32, tag="sq")
                    nc.scalar.activation(
                        sq,
                        img_t[:, i, c, :],
                        AF.Square,
                        accum_out=nsq01[:, col : col + 1],
                    )

        # ---- tail rows (a = 256..263) vs false texts ------------------------
        for f in range(FTN):
            prodf = tmppool.tile([BPC, D], F32, tag="prodf")
            nc.vector.scalar_tensor_tensor(
                out=prodf,
                in0=tail_t[:, f, :],
                scalar=1.0,
                in1=false_t[:, f, :],
                op0=ALU.mult,
                op1=ALU.mult,
                accum_out=dots_f[:, f : f + 1],
            )
            sqf = tmppool.tile([BPC, D], F32, tag="sqf")
            nc.scalar.activation(
                sqf, tail_t[:, f, :], AF.Square, accum_out=nsq_f[:, f : f + 1]
            )
            sqft = tmppool.tile([BPC, D], F32, tag="sqft")
            nc.scalar.activation(
                sqft, false_t[:, f, :], AF.Square, accum_out=nsq_ft[:, f : f + 1]
            )

        # ---- text norms -----------------------------------------------------
        rn_sq = small.tile([128, 2], F32)
        for c in range(2):
            sqr = tmppool.tile([128, D], F32, tag="sqr")
            nc.scalar.activation(
                sqr, rand2[:, c, :], AF.Square, accum_out=rn_sq[:, c : c + 1]
            )

        # ---- normalized, scaled logits --------------------------------------
        # a-chunk block LB[p, c*32+b] = scale * dots / (|img| * |rand|)
        inv01 = small.tile([128, 2 * BPC], F32)
        nc.scalar.activation(inv01, nsq01, AF.Sqrt)
        nc.vector.reciprocal(inv01, inv01)
        rn_isc = small.tile([128, 2], F32)
        nc.scalar.activation(rn_isc, rn_sq, AF.Sqrt)
        nc.vector.reciprocal(rn_isc, rn_isc)
        nc.vector.tensor_scalar_mul(rn_isc, rn_isc, scale_b[:, 0:1])

        LB = small.tile([128, 2 * BPC], F32)
        nc.vector.tensor_mul(LB, dots01, inv01)
        for c in range(2):
            blk = slice(c * BPC, (c + 1) * BPC)
            nc.vector.tensor_scalar_mul(LB[:, blk], LB[:, blk], rn_isc[:, c : c + 1])

        # tail logits, written straight into the row-layout tile
        L_rows = small.tile([BPC, ATN], F32)
        invf = small.tile([BPC, FTN], F32)
        nc.scalar.activation(invf, nsq_f, AF.Sqrt)
        nc.vector.reciprocal(invf, invf)
        invft = small.tile([BPC, FTN], F32)
        nc.scalar.activation(invft, nsq_ft, AF.Sqrt)
        nc.vector.reciprocal(invft, invft)
        lf = small.tile([BPC, FTN], F32)
        nc.vector.tensor_mul(lf, dots_f, invf)
        nc.vector.tensor_mul(lf, lf, invft)
        nc.vector.tensor_scalar_mul(L_rows[:, BS:ATN], lf, scale_b[0:BPC, 0:1])

        # transpose the two [128, 32] chunks into row layout [32, 256]
        for c in range(2):
            pt = psum.tile([BPC, 128], F32, tag="ptr")
            nc.tensor.transpose(pt, LB[:, c * BPC : (c + 1) * BPC], id128)
            nc.scalar.copy(L_rows[:, c * 128 : (c + 1) * 128], pt)

        # ---- per-core row CE partial: sum_b (log sum_a exp(L) - L[b, label_b])
        erow = tmppool.tile([BPC, ATN], F32, tag="erow")
        rs = small.tile([BPC, 1], F32)
        nc.scalar.activation(erow, L_rows, AF.Exp, accum_out=rs)
        lse = small.tile([BPC, 1], F32)
        nc.scalar.activation(lse, rs, AF.Ln)
        dprod = tmppool.tile([BPC, ATN], F32, tag="dprod")
        diag = small.tile([BPC, 1], F32)
        nc.vector.scalar_tensor_tensor(
                out=dprod,
                in0=L_rows,
                scalar=1.0,
                in1=rmask,
                op0=ALU.mult,
                op1=ALU.mult,
                accum_out=diag,
            )
        part = small.tile([BPC, 1], F32)
        nc.vector.tensor_sub(part, lse, diag)
        pp = psum.tile([1, 1], F32, tag="pp")
        nc.tensor.matmul(pp, part, ones128[0:BPC, :], start=True, stop=True)
        ce_sb = small.tile([1, BPC], F32)
        nc.vector.memset(ce_sb, 0.0)
        nc.scalar.copy(ce_sb[:, 0:1], pp)

        # ---- AllGather the [256, 32] logits block + row-CE partial ----------
        payload = dram.tile([2 * 128 + 1, BPC], F32)
        gathered = dram.tile([NCORES * (2 * 128 + 1), BPC], F32)
        nc.sync.dma_start(out=payload[0:128, :], in_=LB[:, 0:BPC])
        nc.sync.dma_start(out=payload[128:256, :], in_=LB[:, BPC : 2 * BPC])
        nc.sync.dma_start(out=payload[256:257, :], in_=ce_sb)
        nc.gpsimd.collective_compute(
            "AllGather",
            ALU.bypass,
            replica_groups=[list(range(NCORES))],
            ins=[payload.opt()],
            outs=[gathered.opt()],
        )

        # ---- transposed CE + final loss (replicated on every core) ----------
        gv = gathered[:, :].rearrange("(m r) j -> r m j", m=NCORES)  # [257, 8, 32]
        pt2 = psum.tile([1, 1], F32, tag="pt2")
        for c in range(2):
            T_c = small.tile([128, NCORES, BPC], F32, tag="tc")
            nc.sync.dma_start(out=T_c, in_=gv[c * 128 : (c + 1) * 128])
            ex = tmppool.tile([128, BS], F32, tag="ex")
            se = small.tile([128, 1], F32, tag="se")
            nc.scalar.activation(ex, T_c, AF.Exp, accum_out=se)
            lz = small.tile([128, 1], F32, tag="lz")
            nc.scalar.activation(lz, se, AF.Ln)
            dg = small.tile([128, 1], F32, tag="dg")
            dgp = tmppool.tile([128, BS], F32, tag="dgp")
            nc.vector.scalar_tensor_tensor(
                out=dgp,
                in0=T_c,
                scalar=1.0,
                in1=eye2[:, c, :],
                op0=ALU.mult,
                op1=ALU.mult,
                accum_out=dg,
            )
            sub = small.tile([128, 1], F32, tag="sub")
            nc.vector.tensor_sub(sub, lz, dg)
            nc.tensor.matmul(pt2, sub, ones128, start=(c == 0), stop=(c == 1))

        # sum of the 8 per-core row-CE partials
        ci = small.tile([1, NCORES], F32)
        nc.sync.dma_start(out=ci, in_=gv[256, :, 0:1])
        cis = small.tile([1, 1], F32)
        nc.vector.reduce_sum(cis, ci, axis=mybir.AxisListType.X)
        tsum = small.tile([1, 1], F32)
        nc.scalar.copy(tsum, pt2)
        tot = small.tile([1, 1], F32)
        nc.vector.tensor_add(tot, tsum, cis)
        res = small.tile([1, 1], F32)
        nc.scalar.mul(res, tot, 1.0 / (2.0 * BS))
        nc.sync.dma_start(out=loss_out[:], in_=res)

    _cap_sync_waits(nc)
    return nc


_NC = None


def _get_nc() -> bass.Bass:
    global _NC
    if _NC is None:
        _NC = build_nc()
    return _NC


def make_in_maps(inputs: dict) -> list[dict]:
    img_full = np.ascontiguousarray(np.asarray(inputs["image_features"], np.float32))
    rand = np.ascontiguousarray(np.asarray(inputs["random_text_features"], np.float32))
    false = np.asarray(inputs["false_text_features"], np.float32).reshape(BS, FTN, D)
    ls = np.asarray(inputs["logit_scale"], np.float32).reshape(1)
    eye = np.eye(BS, dtype=np.float32)
    in_maps = []
    for m in range(NCORES):
        sl = slice(m * BPC, (m + 1) * BPC)
        rm = np.zeros((BPC, ATN), np.float32)
        rm[np.arange(BPC), m * BPC + np.arange(BPC)] = 1.0
        in_maps.append(
            {
                "img": np.ascontiguousarray(img_full[sl]),
                "rand": rand,
                "falset": np.ascontiguousarray(false[sl]),
                "lscale": ls,
                "eye": eye,
                "rowmask": rm,
            }
        )
    return in_maps


def kernel(**inputs) -> np.ndarray:
    nc = _get_nc()
    res = run_bass_kernel_spmd(nc, make_in_maps(inputs), list(range(NCORES)))
    out = np.asarray(res.results[0]["loss_out"], dtype=np.float32)
    return out.reshape(())



# revision 62
# speedup vs baseline: 3.9994x; 3.9994x over previous
"""Trainium2 Bass kernel for the 27092653703365 contrastive loss.

Strategy (memory-bound; ~138 MB of image features dominates):
  - Data-parallel shard of the batch dim (bs=256) across 8 NeuronCores
    (32 images per core); random_text_features replicated.
  - Per core: stream the [32, 256, 512] image block through SBUF once.
    Each (image, text-row) needs only its dot with one text vector and
    its squared norm, so the stream is elementwise work: DVE does the
    dots (scalar_tensor_tensor + accumulate) and the squares are split
    DVE/ACT so both engines stay under the HBM DMA roofline.
  - Tail rows (a=256..263) and their false texts are packed 2-rows-per
    -partition ([128, 2, 512]) and processed before the main loop to
    fill the DMA ramp.
  - The cross-core exchange is a single small AllGather of per-core
    partial column sums of exp(logits) ([256] floats) plus two scalars
    (sum of row-lse, sum of diagonal logits); every core then finishes
    the scalar loss locally.
"""

import sys

sys.path.insert(0, "/opt/trn_rl_repo")

from contextlib import ExitStack

import numpy as np

import concourse.bass as bass
import concourse.tile as tile
from concourse import mybir
from concourse.bass_utils import run_bass_kernel_spmd

F32 = mybir.dt.float32
BF16 = mybir.dt.bfloat16
AF = mybir.ActivationFunctionType
ALU = mybir.AluOpType
AX = mybir.AxisListType

NCORES = 8
BS, FTN, D = 256, 8, 512
ATN = BS + FTN  # 264
BPC = BS // NCORES  # 32 images per core
assert sum([4, 4, 8, 8, 8]) == BPC
# image-group sizes: small first group shortens the DMA ramp; small last
# group lets chunk 0 finish (and its AllGather start) early
GROUPS = [4, 8, 8, 8, 4]
# Square-unit split (cost model: DVE STT ~0.57us/unit, ACT Square
# ~0.87us/unit, no 2x mode for accumulating ops): DVE takes all 72 dot
# units plus 20 of the 80 square units, ACT the other 60 -> ~52us each.


def _cap_sync_waits(nc: bass.Bass, max_waits: int = 1) -> None:
    """The walrus build in this container encodes at most one sync-wait
    command per instruction ("Too many sync wait commands" in codegen
    otherwise), but Tile freely attaches several. Splitting the surplus
    waits onto single-wait Drain carriers right before the instruction is
    semantically identical: the engine blocks on each in turn.
    """
    for func in nc.m.functions:
        for bb in func.blocks:
            out = []
            for ins in bb.instructions:
                si = ins.sync_info
                if si is not None and len(si.on_wait) > max_waits:
                    waits = list(si.on_wait)
                    extra, keep = waits[:-max_waits], waits[-max_waits:]
                    for k, w in enumerate(extra):
                        d = mybir.InstDrain(
                            name=f"{ins.name}_w{k}",
                            ins=[],
                            outs=[],
                            engine=ins.engine,
                        )
                        d.sync_info = mybir.SyncInfo(on_wait=[w], on_update=[])
                        nc.register_instruction(d, overwrite=True)
                        out.append(d)
                    ins.sync_info = mybir.SyncInfo(
                        on_wait=keep, on_update=list(si.on_update)
                    )
                out.append(ins)
            bb.instructions = out


def build_nc() -> bass.Bass:
    nc = bass.Bass(num_devices=NCORES)

    img = nc.declare_dram_parameter("img", [BPC, ATN, D], F32, isOutput=False)
    rand = nc.declare_dram_parameter("rand", [BS, D], F32, isOutput=False)
    falset = nc.declare_dram_parameter("falset", [BPC * FTN, D], F32, isOutput=False)
    lscale = nc.declare_dram_parameter("lscale", [1], F32, isOutput=False)
    ident = nc.declare_dram_parameter("ident", [128, 128], F32, isOutput=False)
    dmask = nc.declare_dram_parameter("dmask", [128, 2 * BPC], F32, isOutput=False)
    loss_out = nc.declare_dram_parameter("loss_out", [1], F32, isOutput=True)

    with tile.TileContext(nc) as tc, ExitStack() as ctx:
        singles = ctx.enter_context(tc.tile_pool(name="singles", bufs=1))
        imgpool = ctx.enter_context(tc.tile_pool(name="img", bufs=2))
        tmppool = ctx.enter_context(tc.tile_pool(name="tmp", bufs=2))
        small = ctx.enter_context(tc.tile_pool(name="small", bufs=2))
        psum = ctx.enter_context(tc.tile_pool(name="psum", bufs=2, space="PSUM"))
        dram = ctx.enter_context(tc.tile_pool(name="dram", bufs=1, space="DRAM"))

        # ---- preloads (ACT HWDGE ring; img stream owns the SP ring) ---------
        ls_raw = singles.tile([128, 1], F32)
        nc.scalar.dma_start(out=ls_raw, in_=lscale[:].to_broadcast([128, 1]))
        # rand text, a-chunked: rand2[p, c, d] = rand[c*128+p, d]
        # (bf16 via SWDGE cast-DMA halves SBUF-side DMA bytes)
        rand2 = singles.tile([128, 2, D], BF16)
        nc.gpsimd.dma_start(out=rand2, in_=rand[:, :].rearrange("(c p) d -> p c d", p=128))
        # 128x128 identity for PE transposes
        id128 = singles.tile([128, 128], F32)
        nc.scalar.dma_start(out=id128, in_=ident[:, :])
        # one-hot mask of this core's diagonal logits in column layout
        dmk = singles.tile([128, 2, BPC], F32)
        nc.scalar.dma_start(
            out=dmk, in_=dmask[:, :].rearrange("p (c b) -> p c b", c=2)
        )

        scale_b = singles.tile([128, 1], F32)
        nc.scalar.activation(scale_b, ls_raw, AF.Exp)
        ones128 = singles.tile([128, 1], F32)
        nc.vector.memset(ones128, 1.0)
        neg2 = singles.tile([128, 1], F32)
        nc.vector.memset(neg2, -2.0)

        # accumulators
        dots01 = singles.tile([128, 2, BPC], F32)
        nsq01 = singles.tile([128, 2, BPC], F32)

        # rand norms (ACT is free while the first img DMA streams)
        rn_sq = small.tile([128, 2], F32)
        for c in range(2):
            sqr = tmppool.tile([128, D], F32, tag="sqr")
            nc.scalar.activation(
                sqr, rand2[:, c, :], AF.Square, accum_out=rn_sq[:, c : c + 1]
            )
        rn_isc = small.tile([128, 2], F32)
        nc.scalar.activation(rn_isc, rn_sq, AF.Ln)
        nc.scalar.activation(rn_isc, rn_isc, AF.Exp, scale=-0.5)
        nc.vector.tensor_scalar_mul(rn_isc, rn_isc, scale_b)

        # persistent logits state (written chunk by chunk)
        inv01 = singles.tile([128, 2, BPC], F32)
        LB = singles.tile([128, 2, BPC], F32)
        expLB = singles.tile([128, 2, BPC], F32)
        cs = singles.tile([128, 2], F32)
        rs = singles.tile([BPC, 1], F32)

        # ---- main stream (chunk-major): dots + squared norms ----------------
        # Chunk 0 (texts a<128) streams first; its logits post-processing then
        # hides under chunk 1's stream.
        #
        # Engine split across the 152 elementwise units (cost model: DVE STT
        # ~0.57us, ACT Square ~0.87us; GPSIMD has no STT opcode on real HW):
        # DVE takes all 72 dots + ~20 squares, ACT ~60 squares -> ~54us each.
        pending_pool = []

        def flush_pool():
            for thunk in pending_pool:
                thunk()
            pending_pool.clear()

        # (dot_engine, square_engine) per image slot within an 8-slot block
        PAIRS = ["DA", "DA", "DD", "DA", "DA", "DD", "DA", "DA"]

        def dve_stt(in0, in1, acc):
            o = tmppool.tile([128, D], BF16, tag="sqd")
            nc.vector.scalar_tensor_tensor(
                out=o, in0=in0, scalar=1.0, in1=in1,
                op0=ALU.mult, op1=ALU.mult, accum_out=acc,
            )

        # alternate img groups between the SP HWDGE ring (f32) and the Pool
        # SWDGE ring (bf16 cast): halves each path's serial transfer load and
        # decouples supply from the Pool queue's compute hazards
        grp_ctr = [0]

        def do_cgroup(c, b0, gsz, no_pool=False):
            src = img[b0 : b0 + gsz, c * 128 : (c + 1) * 128, :].rearrange(
                "g p d -> p g d"
            )
            if grp_ctr[0] % 2 == 0:
                img_t = imgpool.tile([128, gsz, D], F32, tag=f"imgf{gsz}")
                nc.sync.dma_start(out=img_t, in_=src)
            else:
                img_t = imgpool.tile([128, gsz, D], BF16, tag=f"imgb{gsz}")
                nc.gpsimd.dma_start(out=img_t, in_=src)
            grp_ctr[0] += 1
            flush_pool()
            for i in range(gsz):
                b = b0 + i
                de, se = PAIRS[b % 8]
                ia = img_t[:, i, :]
                ra = rand2[:, c, :]
                dacc = dots01[:, c, b : b + 1]
                sacc = nsq01[:, c, b : b + 1]
                dve_stt(ia, ra, dacc)
                if se == "A":
                    sqa = tmppool.tile([128, D], BF16, tag="sqa")
                    nc.scalar.activation(sqa, ia, AF.Square, accum_out=sacc)
                else:
                    dve_stt(ia, ia, sacc)

        dcol = singles.tile([128, 2], F32)

        def post_chunk(c):
            nc.scalar.activation(inv01[:, c, :], nsq01[:, c, :], AF.Ln)
            nc.scalar.activation(inv01[:, c, :], inv01[:, c, :], AF.Exp, scale=-0.5)
            nc.vector.tensor_mul(LB[:, c, :], dots01[:, c, :], inv01[:, c, :])
            nc.vector.tensor_scalar_mul(
                LB[:, c, :], LB[:, c, :], rn_isc[:, c : c + 1]
            )
            nc.scalar.activation(expLB[:, c, :], LB[:, c, :], AF.Exp)
            nc.vector.tensor_reduce(
                cs[:, c : c + 1], expLB[:, c, :], axis=AX.X, op=ALU.add
            )
            # this chunk's share of the diagonal partial
            dprod = tmppool.tile([128, BPC], F32, tag="dprod")
            nc.vector.scalar_tensor_tensor(
                out=dprod,
                in0=LB[:, c, :],
                scalar=1.0,
                in1=dmk[:, c, :],
                op0=ALU.mult,
                op1=ALU.mult,
                accum_out=dcol[:, c : c + 1],
            )
            pt = psum.tile([BPC, 128], F32, tag="ptr")
            nc.tensor.transpose(pt, expLB[:, c, :], id128)
            rc = small.tile([BPC, 1], F32, tag="rc")
            nc.vector.tensor_reduce(rc, pt, axis=AX.X, op=ALU.add)
            if c == 0:
                nc.vector.tensor_add(rs, rc, rst)
            else:
                nc.vector.tensor_add(rs, rs, rc)

        # first (small) group before the tail block: compute starts ASAP
        do_cgroup(0, 0, GROUPS[0])

        # false texts + image tail rows, both [b=32, f=8, d] (after the first
        # img group in the SWDGE queue so dots start ASAP)
        false_t = singles.tile([BPC, FTN, D], BF16)
        nc.gpsimd.dma_start(
            out=false_t, in_=falset[:, :].rearrange("(b f) d -> b f d", f=FTN)
        )
        tail_t = singles.tile([BPC, FTN, D], BF16)
        nc.gpsimd.dma_start(out=tail_t, in_=img[:, BS:ATN, :])

        # ---- tail rows vs false texts (overlaps the img stream) -------------
        ltr = small.tile([BPC, FTN], F32)
        nsq_t = small.tile([BPC, FTN], F32)
        nsq_f = small.tile([BPC, FTN], F32)
        tsq_unit = 0
        for f in range(FTN):
            prodf = tmppool.tile([BPC, D], BF16, tag="prodf")
            nc.vector.scalar_tensor_tensor(
                out=prodf,
                in0=tail_t[:, f, :],
                scalar=1.0,
                in1=false_t[:, f, :],
                op0=ALU.mult,
                op1=ALU.mult,
                accum_out=ltr[:, f : f + 1],
            )
            for src_t, acc in ((tail_t, nsq_t), (false_t, nsq_f)):
                if tsq_unit % 4 != 3:
                    sq = tmppool.tile([BPC, D], BF16, tag="tsqa")
                    nc.scalar.activation(
                        sq, src_t[:, f, :], AF.Square, accum_out=acc[:, f : f + 1]
                    )
                else:
                    sq = tmppool.tile([BPC, D], BF16, tag="tsqd")
                    nc.vector.scalar_tensor_tensor(
                        out=sq,
                        in0=src_t[:, f, :],
                        scalar=1.0,
                        in1=src_t[:, f, :],
                        op0=ALU.mult,
                        op1=ALU.mult,
                        accum_out=acc[:, f : f + 1],
                    )
                tsq_unit += 1
        flush_pool()  # the deferred tail squares must land before the Ln reads
        # 1/sqrt(x) = exp(-0.5*ln(x)): keeps ACT on the natural_log_exp
        # table set (Square/Exp/Ln/Copy coexist there; Sqrt would force a
        # ~1.3us table reload per switch)
        inv_t = small.tile([BPC, FTN], F32)
        nc.scalar.activation(inv_t, nsq_t, AF.Ln)
        nc.scalar.activation(inv_t, inv_t, AF.Exp, scale=-0.5)
        inv_f = small.tile([BPC, FTN], F32)
        nc.scalar.activation(inv_f, nsq_f, AF.Ln)
        nc.scalar.activation(inv_f, inv_f, AF.Exp, scale=-0.5)
        lt = small.tile([BPC, FTN], F32)
        nc.vector.tensor_mul(lt, ltr, inv_t)
        nc.vector.tensor_mul(lt, lt, inv_f)
        nc.vector.tensor_scalar_mul(lt, lt, scale_b[0:BPC, :])
        exp_t = small.tile([BPC, FTN], F32)
        nc.scalar.activation(exp_t, lt, AF.Exp)
        # per-image tail exp sum [32, 1]
        rst = small.tile([BPC, 1], F32)
        nc.vector.tensor_reduce(rst, exp_t, axis=AX.X, op=ALU.add)

        # ---- remaining chunk-0 groups, then chunk 0 post ---------------------
        b0 = GROUPS[0]
        for gi, gsz in enumerate(GROUPS[1:], start=1):
            # the last chunk-0 group avoids deferred Pool units so chunk 0's
            # accumulators complete (and AllGather-0 can fire) early
            do_cgroup(0, b0, gsz, no_pool=(gi == len(GROUPS) - 1))
            b0 += gsz

        # ---- chunk 1 stream; chunk 0 post + payload write hide under it ------
        payload = dram.tile([1, 2 * 128 + 1], F32)
        gathered = dram.tile([NCORES, 2 * 128 + 1], F32)
        b0 = 0
        for gi, gsz in enumerate(GROUPS):
            do_cgroup(1, b0, gsz)
            b0 += gsz
            if gi == 0:
                flush_pool()
                post_chunk(0)
                nc.sync.dma_start(
                    out=payload[0:1, 0:128].rearrange("o p -> p o"), in_=cs[:, 0:1]
                )
        flush_pool()
        post_chunk(1)

        lse = small.tile([BPC, 1], F32)
        nc.scalar.activation(lse, rs, AF.Ln)
        dsum = small.tile([128, 1], F32)
        nc.vector.tensor_add(dsum, dcol[:, 0:1], dcol[:, 1:2])

        # u = sum_i lse_i - 2 * sum diag  (single PSUM accumulation)
        u_ps = psum.tile([1, 1], F32, tag="usum")
        nc.tensor.matmul(u_ps, dsum, neg2, start=True, stop=False)
        nc.tensor.matmul(u_ps, lse, ones128[0:BPC, :], start=False, stop=True)
        uv2 = small.tile([1, 1], F32)
        nc.scalar.copy(uv2, u_ps)

        # ---- AllGather: colsum partials (both chunks) + row-CE partial ------
        nc.sync.dma_start(
            out=payload[0:1, 128:256].rearrange("o p -> p o"), in_=cs[:, 1:2]
        )
        nc.sync.dma_start(out=payload[0:1, 256:257], in_=uv2)
        nc.gpsimd.collective_compute(
            "AllGather",
            ALU.bypass,
            replica_groups=[list(range(NCORES))],
            ins=[payload.opt()],
            outs=[gathered.opt()],
        )

        # ---- finish the loss (replicated on every core) ---------------------
        csg = small.tile([128, 2, NCORES], F32)
        for c in range(2):
            nc.sync.dma_start(
                out=csg[:, c, :],
                in_=gathered[:, c * 128 : (c + 1) * 128].rearrange("m p -> p m"),
            )
        sc = small.tile([1, NCORES], F32)
        nc.sync.dma_start(
            out=sc, in_=gathered[:, 256:257].rearrange("m k -> k m")
        )
        cst = small.tile([128, 2], F32)
        nc.vector.tensor_reduce(cst, csg, axis=AX.X, op=ALU.add)
        lncs = small.tile([128, 2], F32)
        nc.scalar.activation(lncs, cst, AF.Ln)
        l1 = small.tile([128, 1], F32)
        nc.vector.tensor_reduce(l1, lncs, axis=AX.X, op=ALU.add)
        l_ps = psum.tile([1, 1], F32, tag="lsum")
        nc.tensor.matmul(l_ps, l1, ones128, start=True, stop=True)

        ut = small.tile([1, 1], F32)
        nc.vector.tensor_reduce(ut, sc, axis=AX.X, op=ALU.add)
        t3 = small.tile([1, 1], F32)
        nc.scalar.copy(t3, l_ps)
        tot = small.tile([1, 1], F32)
        nc.vector.tensor_add(tot, ut, t3)
        res = small.tile([1, 1], F32)
        nc.scalar.mul(res, tot, 1.0 / (2.0 * BS))
        nc.sync.dma_start(out=loss_out[:], in_=res)

    _cap_sync_waits(nc)
    return nc


_NC = None


def _get_nc() -> bass.Bass:
    global _NC
    if _NC is None:
        _NC = build_nc()
    return _NC


def make_in_maps(inputs: dict) -> list[dict]:
    img_full = np.ascontiguousarray(np.asarray(inputs["image_features"], np.float32))
    rand = np.ascontiguousarray(np.asarray(inputs["random_text_features"], np.float32))
    false = np.asarray(inputs["false_text_features"], np.float32)
    ls = np.asarray(inputs["logit_scale"], np.float32).reshape(1)
    ident = np.eye(128, dtype=np.float32)
    in_maps = []
    for m in range(NCORES):
        sl = slice(m * BPC, (m + 1) * BPC)
        dm = np.zeros((128, 2 * BPC), np.float32)
        a = m * BPC + np.arange(BPC)
        dm[a % 128, (a // 128) * BPC + np.arange(BPC)] = 1.0
        in_maps.append(
            {
                "img": np.ascontiguousarray(img_full[sl]),
                "rand": rand,
                "falset": np.ascontiguousarray(false[m * BPC * FTN : (m + 1) * BPC * FTN]),
                "lscale": ls,
                "ident": ident,
                "dmask": dm,
            }
        )
    return in_maps


def kernel(**inputs) -> np.ndarray:
    nc = _get_nc()
    res = run_bass_kernel_spmd(nc, make_in_maps(inputs), list(range(NCORES)))
    out = np.asarray(res.results[0]["loss_out"], dtype=np.float32)
    return out.reshape(())


# revision 69
# speedup vs baseline: 6.7053x; 1.6766x over previous
"""Trainium2 Bass kernel for the 27092653703365 contrastive loss.

Strategy (memory-bound; ~138 MB of image features dominates):
  - Data-parallel shard of the batch dim (bs=256) across 8 NeuronCores
    (32 images per core); random_text_features replicated.
  - Per core: stream the [32, 256, 512] image block through SBUF once.
    Each (image, text-row) needs only its dot with one text vector and
    its squared norm, so the stream is elementwise work: DVE does the
    dots (scalar_tensor_tensor + accumulate) and the squares are split
    DVE/ACT so both engines stay under the HBM DMA roofline.
  - Tail rows (a=256..263) vs false texts are processed early so their
    compute fills the DMA ramp; img groups alternate between the SP
    HWDGE ring (f32) and the Pool SWDGE ring (bf16 cast) to split the
    transfer load; all activations stay on one ACT table set (1/sqrt
    computed as exp(-0.5 ln)).
  - The cross-core exchange is a single small AllGather of per-core
    partial column sums of exp(logits) ([256] floats) plus one scalar
    (sum of row-lse minus twice the diagonal sum); every core then
    finishes the scalar loss locally.
"""

import sys

sys.path.insert(0, "/opt/trn_rl_repo")

from contextlib import ExitStack

import numpy as np

import concourse.bass as bass
import concourse.tile as tile
from concourse import mybir
from concourse.bass_utils import run_bass_kernel_spmd

F32 = mybir.dt.float32
BF16 = mybir.dt.bfloat16
AF = mybir.ActivationFunctionType
ALU = mybir.AluOpType
AX = mybir.AxisListType

NCORES = 8
BS, FTN, D = 256, 8, 512
ATN = BS + FTN  # 264
BPC = BS // NCORES  # 32 images per core
# image-group sizes: small first group shortens the DMA ramp; small last
# group lets chunk 0 finish (and its post-processing start) early
GROUPS = [4, 8, 8, 8, 4]
assert sum(GROUPS) == BPC


def _cap_sync_waits(nc: bass.Bass, max_waits: int = 1) -> None:
    """The walrus build in this container encodes at most one sync-wait
    command per instruction ("Too many sync wait commands" in codegen
    otherwise), but Tile freely attaches several. Splitting the surplus
    waits onto single-wait Drain carriers right before the instruction is
    semantically identical: the engine blocks on each in turn.
    """
    for func in nc.m.functions:
        for bb in func.blocks:
            out = []
            for ins in bb.instructions:
                si = ins.sync_info
                if si is not None and len(si.on_wait) > max_waits:
                    waits = list(si.on_wait)
                    extra, keep = waits[:-max_waits], waits[-max_waits:]
                    for k, w in enumerate(extra):
                        d = mybir.InstDrain(
                            name=f"{ins.name}_w{k}",
                            ins=[],
                            outs=[],
                            engine=ins.engine,
                        )
                        d.sync_info = mybir.SyncInfo(on_wait=[w], on_update=[])
                        nc.register_instruction(d, overwrite=True)
                        out.append(d)
                    ins.sync_info = mybir.SyncInfo(
                        on_wait=keep, on_update=list(si.on_update)
                    )
                out.append(ins)
            bb.instructions = out


def build_nc() -> bass.Bass:
    nc = bass.Bass(num_devices=NCORES)

    img = nc.declare_dram_parameter("img", [BPC, ATN, D], F32, isOutput=False)
    rand = nc.declare_dram_parameter("rand", [BS, D], F32, isOutput=False)
    falset = nc.declare_dram_parameter("falset", [BPC * FTN, D], F32, isOutput=False)
    lscale = nc.declare_dram_parameter("lscale", [1], F32, isOutput=False)
    ident = nc.declare_dram_parameter("ident", [128, 128], F32, isOutput=False)
    dmask = nc.declare_dram_parameter("dmask", [128, 2 * BPC], F32, isOutput=False)
    loss_out = nc.declare_dram_parameter("loss_out", [1], F32, isOutput=True)

    with tile.TileContext(nc) as tc, ExitStack() as ctx:
        singles = ctx.enter_context(tc.tile_pool(name="singles", bufs=1))
        imgpool = ctx.enter_context(tc.tile_pool(name="img", bufs=2))
        tmppool = ctx.enter_context(tc.tile_pool(name="tmp", bufs=2))
        small = ctx.enter_context(tc.tile_pool(name="small", bufs=2))
        psum = ctx.enter_context(tc.tile_pool(name="psum", bufs=2, space="PSUM"))
        dram = ctx.enter_context(tc.tile_pool(name="dram", bufs=1, space="DRAM"))

        # ---- preloads (ACT HWDGE ring; img stream owns the SP ring) ---------
        ls_raw = singles.tile([128, 1], F32)
        nc.scalar.dma_start(out=ls_raw, in_=lscale[:].to_broadcast([128, 1]))
        # rand text, a-chunked: rand2[p, c, d] = rand[c*128+p, d]
        # (bf16 via SWDGE cast-DMA halves SBUF-side DMA bytes)
        rand2 = singles.tile([128, 2, D], BF16)
        nc.gpsimd.dma_start(out=rand2, in_=rand[:, :].rearrange("(c p) d -> p c d", p=128))
        # 128x128 identity for PE transposes
        id128 = singles.tile([128, 128], F32)
        nc.scalar.dma_start(out=id128, in_=ident[:, :])
        # one-hot mask of this core's diagonal logits in column layout
        dmk = singles.tile([128, 2, BPC], F32)
        nc.scalar.dma_start(
            out=dmk, in_=dmask[:, :].rearrange("p (c b) -> p c b", c=2)
        )

        scale_b = singles.tile([128, 1], F32)
        nc.scalar.activation(scale_b, ls_raw, AF.Exp)
        ones128 = singles.tile([128, 1], F32)
        nc.vector.memset(ones128, 1.0)
        neg2 = singles.tile([128, 1], F32)
        nc.vector.memset(neg2, -2.0)

        # accumulators
        dots01 = singles.tile([128, 2, BPC], F32)
        nsq01 = singles.tile([128, 2, BPC], F32)

        # rand norms (ACT is free while the first img DMA streams)
        rn_sq = small.tile([128, 2], F32)
        for c in range(2):
            sqr = tmppool.tile([128, D], F32, tag="sqr")
            nc.scalar.activation(
                sqr, rand2[:, c, :], AF.Square, accum_out=rn_sq[:, c : c + 1]
            )
        rn_isc = small.tile([128, 2], F32)
        nc.scalar.activation(rn_isc, rn_sq, AF.Ln)
        nc.scalar.activation(rn_isc, rn_isc, AF.Exp, scale=-0.5)
        nc.vector.tensor_scalar_mul(rn_isc, rn_isc, scale_b)

        # persistent logits state (written chunk by chunk)
        inv01 = singles.tile([128, 2, BPC], F32)
        LB = singles.tile([128, 2, BPC], F32)
        expLB = singles.tile([128, 2, BPC], F32)
        cs = singles.tile([128, 2], F32)
        rs = singles.tile([BPC, 1], F32)

        # ---- main stream (chunk-major): dots + squared norms ----------------
        # Chunk 0 (texts a<128) streams first; its logits post-processing then
        # hides under chunk 1's stream.
        #
        # Engine split across the 152 elementwise units (cost model: DVE STT
        # ~0.57us, ACT Square ~0.87us; GPSIMD has no STT opcode on real HW):
        # DVE takes all 72 dots + ~20 squares, ACT ~60 squares -> ~54us each.
        # square_engine per image slot within an 8-slot block
        SQ_ENG = ["A", "A", "D", "A", "A", "D", "A", "A"]

        def dve_stt(in0, in1, acc):
            o = tmppool.tile([128, D], BF16, tag="sqd")
            nc.vector.scalar_tensor_tensor(
                out=o, in0=in0, scalar=1.0, in1=in1,
                op0=ALU.mult, op1=ALU.mult, accum_out=acc,
            )

        # alternate img groups between the SP HWDGE ring (f32) and the Pool
        # SWDGE ring (bf16 cast): splits the transfer load across both DMA
        # paths and doubles the effective prefetch depth
        grp_ctr = [0]

        def do_cgroup(c, b0, gsz):
            src = img[b0 : b0 + gsz, c * 128 : (c + 1) * 128, :].rearrange(
                "g p d -> p g d"
            )
            if grp_ctr[0] % 2 == 0:
                img_t = imgpool.tile([128, gsz, D], F32, tag=f"imgf{gsz}")
                nc.sync.dma_start(out=img_t, in_=src)
            else:
                img_t = imgpool.tile([128, gsz, D], BF16, tag=f"imgb{gsz}")
                nc.gpsimd.dma_start(out=img_t, in_=src)
            grp_ctr[0] += 1
            for i in range(gsz):
                b = b0 + i
                se = SQ_ENG[b % 8]
                ia = img_t[:, i, :]
                ra = rand2[:, c, :]
                dacc = dots01[:, c, b : b + 1]
                sacc = nsq01[:, c, b : b + 1]
                dve_stt(ia, ra, dacc)
                if se == "A":
                    sqa = tmppool.tile([128, D], BF16, tag="sqa")
                    nc.scalar.activation(sqa, ia, AF.Square, accum_out=sacc)
                else:
                    dve_stt(ia, ia, sacc)

        dcol = singles.tile([128, 2], F32)

        def post_chunk(c):
            nc.scalar.activation(inv01[:, c, :], nsq01[:, c, :], AF.Ln)
            nc.scalar.activation(inv01[:, c, :], inv01[:, c, :], AF.Exp, scale=-0.5)
            nc.vector.tensor_mul(LB[:, c, :], dots01[:, c, :], inv01[:, c, :])
            nc.vector.tensor_scalar_mul(
                LB[:, c, :], LB[:, c, :], rn_isc[:, c : c + 1]
            )
            nc.scalar.activation(expLB[:, c, :], LB[:, c, :], AF.Exp)
            nc.vector.tensor_reduce(
                cs[:, c : c + 1], expLB[:, c, :], axis=AX.X, op=ALU.add
            )
            # this chunk's share of the diagonal partial
            dprod = tmppool.tile([128, BPC], F32, tag="dprod")
            nc.vector.scalar_tensor_tensor(
                out=dprod,
                in0=LB[:, c, :],
                scalar=1.0,
                in1=dmk[:, c, :],
                op0=ALU.mult,
                op1=ALU.mult,
                accum_out=dcol[:, c : c + 1],
            )
            pt = psum.tile([BPC, 128], F32, tag="ptr")
            nc.tensor.transpose(pt, expLB[:, c, :], id128)
            rc = small.tile([BPC, 1], F32, tag="rc")
            nc.vector.tensor_reduce(rc, pt, axis=AX.X, op=ALU.add)
            if c == 0:
                nc.vector.tensor_add(rs, rc, rst)
            else:
                nc.vector.tensor_add(rs, rs, rc)

        # first (small) group before the tail block: compute starts ASAP
        do_cgroup(0, 0, GROUPS[0])

        # false texts + image tail rows, both [b=32, f=8, d] (after the first
        # img group in the SWDGE queue so dots start ASAP)
        false_t = singles.tile([BPC, FTN, D], BF16)
        nc.gpsimd.dma_start(
            out=false_t, in_=falset[:, :].rearrange("(b f) d -> b f d", f=FTN)
        )
        tail_t = singles.tile([BPC, FTN, D], BF16)
        nc.gpsimd.dma_start(out=tail_t, in_=img[:, BS:ATN, :])

        # ---- tail rows vs false texts (overlaps the img stream) -------------
        ltr = small.tile([BPC, FTN], F32)
        nsq_t = small.tile([BPC, FTN], F32)
        nsq_f = small.tile([BPC, FTN], F32)
        tsq_unit = 0
        for f in range(FTN):
            prodf = tmppool.tile([BPC, D], BF16, tag="prodf")
            nc.vector.scalar_tensor_tensor(
                out=prodf,
                in0=tail_t[:, f, :],
                scalar=1.0,
                in1=false_t[:, f, :],
                op0=ALU.mult,
                op1=ALU.mult,
                accum_out=ltr[:, f : f + 1],
            )
            for src_t, acc in ((tail_t, nsq_t), (false_t, nsq_f)):
                if tsq_unit % 4 != 3:
                    sq = tmppool.tile([BPC, D], BF16, tag="tsqa")
                    nc.scalar.activation(
                        sq, src_t[:, f, :], AF.Square, accum_out=acc[:, f : f + 1]
                    )
                else:
                    sq = tmppool.tile([BPC, D], BF16, tag="tsqd")
                    nc.vector.scalar_tensor_tensor(
                        out=sq,
                        in0=src_t[:, f, :],
                        scalar=1.0,
                        in1=src_t[:, f, :],
                        op0=ALU.mult,
                        op1=ALU.mult,
                        accum_out=acc[:, f : f + 1],
                    )
                tsq_unit += 1
        # 1/sqrt(x) = exp(-0.5*ln(x)): keeps ACT on the natural_log_exp
        # table set (Square/Exp/Ln/Copy coexist there; Sqrt would force a
        # ~1.3us table reload per switch)
        inv_t = small.tile([BPC, FTN], F32)
        nc.scalar.activation(inv_t, nsq_t, AF.Ln)
        nc.scalar.activation(inv_t, inv_t, AF.Exp, scale=-0.5)
        inv_f = small.tile([BPC, FTN], F32)
        nc.scalar.activation(inv_f, nsq_f, AF.Ln)
        nc.scalar.activation(inv_f, inv_f, AF.Exp, scale=-0.5)
        lt = small.tile([BPC, FTN], F32)
        nc.vector.tensor_mul(lt, ltr, inv_t)
        nc.vector.tensor_mul(lt, lt, inv_f)
        nc.vector.tensor_scalar_mul(lt, lt, scale_b[0:BPC, :])
        exp_t = small.tile([BPC, FTN], F32)
        nc.scalar.activation(exp_t, lt, AF.Exp)
        # per-image tail exp sum [32, 1]
        rst = small.tile([BPC, 1], F32)
        nc.vector.tensor_reduce(rst, exp_t, axis=AX.X, op=ALU.add)

        # ---- remaining chunk-0 groups, then chunk 0 post ---------------------
        b0 = GROUPS[0]
        for gsz in GROUPS[1:]:
            do_cgroup(0, b0, gsz)
            b0 += gsz

        # ---- chunk 1 stream; chunk 0 post + payload write hide under it ------
        payload = dram.tile([1, 2 * 128 + 1], F32)
        gathered = dram.tile([NCORES, 2 * 128 + 1], F32)
        b0 = 0
        for gi, gsz in enumerate(GROUPS):
            do_cgroup(1, b0, gsz)
            b0 += gsz
            if gi == 0:
                post_chunk(0)
                nc.sync.dma_start(
                    out=payload[0:1, 0:128].rearrange("o p -> p o"), in_=cs[:, 0:1]
                )
        post_chunk(1)

        lse = small.tile([BPC, 1], F32)
        nc.scalar.activation(lse, rs, AF.Ln)
        dsum = small.tile([128, 1], F32)
        nc.vector.tensor_add(dsum, dcol[:, 0:1], dcol[:, 1:2])

        # u = sum_i lse_i - 2 * sum diag  (single PSUM accumulation)
        u_ps = psum.tile([1, 1], F32, tag="usum")
        nc.tensor.matmul(u_ps, dsum, neg2, start=True, stop=False)
        nc.tensor.matmul(u_ps, lse, ones128[0:BPC, :], start=False, stop=True)
        uv2 = small.tile([1, 1], F32)
        nc.scalar.copy(uv2, u_ps)

        # ---- AllGather: colsum partials (both chunks) + row-CE partial ------
        nc.sync.dma_start(
            out=payload[0:1, 128:256].rearrange("o p -> p o"), in_=cs[:, 1:2]
        )
        nc.sync.dma_start(out=payload[0:1, 256:257], in_=uv2)
        nc.gpsimd.collective_compute(
            "AllGather",
            ALU.bypass,
            replica_groups=[list(range(NCORES))],
            ins=[payload.opt()],
            outs=[gathered.opt()],
        )

        # ---- finish the loss (replicated on every core) ---------------------
        csg = small.tile([128, 2, NCORES], F32)
        for c in range(2):
            nc.sync.dma_start(
                out=csg[:, c, :],
                in_=gathered[:, c * 128 : (c + 1) * 128].rearrange("m p -> p m"),
            )
        sc = small.tile([1, NCORES], F32)
        nc.sync.dma_start(
            out=sc, in_=gathered[:, 256:257].rearrange("m k -> k m")
        )
        cst = small.tile([128, 2], F32)
        nc.vector.tensor_reduce(cst, csg, axis=AX.X, op=ALU.add)
        lncs = small.tile([128, 2], F32)
        nc.scalar.activation(lncs, cst, AF.Ln)
        l1 = small.tile([128, 1], F32)
        nc.vector.tensor_reduce(l1, lncs, axis=AX.X, op=ALU.add)
        l_ps = psum.tile([1, 1], F32, tag="lsum")
        nc.tensor.matmul(l_ps, l1, ones128, start=True, stop=True)

        ut = small.tile([1, 1], F32)
        nc.vector.tensor_reduce(ut, sc, axis=AX.X, op=ALU.add)
        t3 = small.tile([1, 1], F32)
        nc.scalar.copy(t3, l_ps)
        tot = small.tile([1, 1], F32)
        nc.vector.tensor_add(tot, ut, t3)
        res = small.tile([1, 1], F32)
        nc.scalar.mul(res, tot, 1.0 / (2.0 * BS))
        nc.sync.dma_start(out=loss_out[:], in_=res)

    _cap_sync_waits(nc)
    return nc


_NC = None


def _get_nc() -> bass.Bass:
    global _NC
    if _NC is None:
        _NC = build_nc()
    return _NC


def make_in_maps(inputs: dict) -> list[dict]:
    img_full = np.ascontiguousarray(np.asarray(inputs["image_features"], np.float32))
    rand = np.ascontiguousarray(np.asarray(inputs["random_text_features"], np.float32))
    false = np.asarray(inputs["false_text_features"], np.float32)
    ls = np.asarray(inputs["logit_scale"], np.float32).reshape(1)
    ident = np.eye(128, dtype=np.float32)
    in_maps = []
    for m in range(NCORES):
        sl = slice(m * BPC, (m + 1) * BPC)
        dm = np.zeros((128, 2 * BPC), np.float32)
        a = m * BPC + np.arange(BPC)
        dm[a % 128, (a // 128) * BPC + np.arange(BPC)] = 1.0
        in_maps.append(
            {
                "img": np.ascontiguousarray(img_full[sl]),
                "rand": rand,
                "falset": np.ascontiguousarray(false[m * BPC * FTN : (m + 1) * BPC * FTN]),
                "lscale": ls,
                "ident": ident,
                "dmask": dm,
            }
        )
    return in_maps


def kernel(**inputs) -> np.ndarray:
    nc = _get_nc()
    res = run_bass_kernel_spmd(nc, make_in_maps(inputs), list(range(NCORES)))
    out = np.asarray(res.results[0]["loss_out"], dtype=np.float32)
    return out.reshape(())
